# revision 1
# baseline (speedup 1.0000x reference)
"""Causal MQA kernel for Trainium2, SPMD over 8 NeuronCores.

Sharding: core i = (batch b = i//4, head-group hg = i%4). Each core computes
the kv projection for its batch (replicated 4x instead of 8x), the q
projection for its 4 heads, and causal attention for those heads over its
batch; it writes the [T, 512] output slice y[b, :, hg*512:(hg+1)*512]. The
host concatenates slices (no device collectives).

Device algorithm (per core, T processed in 4 chunks of QC=512 queries):
  - x arrives transposed and pre-cast to fp16 (xT = x[b].T, [C, T]); the
    projections emit kT/vT/qT in [head_dim, T] fp16 layout directly.
  - S^T[k, q] = matmul(lhsT=kT_tile, rhs=qT_chunk) in fp16 (fp32 PSUM).
  - P^T = exp(S^T / sqrt(hd)), no max-subtraction (scores are O(1) for this
    problem's 0.02-scaled weights); causal mask applied multiplicatively on
    diagonal tiles only, with matmul/exp/mask restricted to the q >= key
    column range (triangle tightening: diagonal tile du covers q >= du*128).
  - y^T [d, q] accumulates in PSUM via matmul(lhsT=V_tile [keys, d], rhs=P^T).
  - softmax denominators: P^T tiles are accumulated across key tiles on the
    vector engine (fp16, 2x rate) into acc [128, 512]; a single
    matmul(lhsT=ones [128,1], rhs=acc) yields sums [1, 512] per chunk-head
    (instead of one [1,512] matmul per key tile - those cost a full 512-col
    PE stream each).
  - tail (deferred one chunk): transpose sums to [128, qt, h] so the
    reciprocal runs across all DVE lanes, transpose y^T via PE, and fold the
    normalization into per-partition ACT scales on the PSUM->SBUF copies;
    one DMA per chunk writes [128, qt, head, d] with 2KB lines.
"""

import math
from contextlib import ExitStack

import numpy as np

import concourse.bass as bass
import concourse.mybir as mybir
import concourse.tile as tile
from concourse import bacc
from concourse.bass_utils import run_bass_kernel_spmd
from concourse.masks import make_identity

F32 = mybir.dt.float32
F16 = mybir.dt.float16
P = 128  # partitions
HD = 128  # head dim
QC = 512  # query-chunk width (one fp32 PSUM bank)
KGRP = 2  # key tiles per score/exp group
N_CORES = 8
HPC = 4  # query heads per core
NB = 4  # head groups (cores per batch)

PHASE_MARKS = []


def _mark(nc, name):
    n = int(nc.get_next_instruction_name().split("-")[-1])
    PHASE_MARKS.append((n, name))


def build_nc(T, C):
    NQC = T // QC  # query chunks (4)
    NCC = C // P  # contraction chunks (16)
    KTQ = QC // P  # key tiles per query chunk (4)
    NS2 = (HPC - 1) * 32 + 2  # 98: sums rows at h*32 (SBUF partition
    # access must start at 32-boundaries), 4B-aligned PSUM transposes
    inv_sqrt_hd = 1.0 / math.sqrt(HD)

    nc = bacc.Bacc("TRN2", target_bir_lowering=False, debug=False,
                   num_devices=N_CORES)
    xT = nc.dram_tensor("xT", [C, T], F16, kind="ExternalInput").ap()
    # per-core slice of xT for this core's kv shard (the SPMD program is
    # identical on every core, so the slice selection happens on the host)
    xkv = nc.dram_tensor("xkv", [C, QC], F16, kind="ExternalInput").ap()
    wq_t = nc.dram_tensor("wq_t", [C, HPC * HD], F16, kind="ExternalInput").ap()
    wkv_t = nc.dram_tensor("wkv_t", [C, 2 * HD], F16, kind="ExternalInput").ap()
    y = nc.dram_tensor("y", [T, HPC * HD], F32, kind="ExternalOutput").ap()
    # kv shard staging for the 4-core (same batch) all-gather:
    # [0] = kT slice [d, 512], [1] = v slice [tok%128, tile, d]
    kv_loc = nc.dram_tensor("kv_loc", [2, P, QC], F16).ap()
    kv_all = nc.dram_tensor("kv_all", [NB, 2, P, QC], F16).ap()

    with tile.TileContext(nc) as tc, ExitStack() as ctx, \
            nc.allow_low_precision(reason="fp16 operands feed the PE (10-bit mantissa); accumulation stays fp32 in PSUM"):
        consts = ctx.enter_context(tc.tile_pool(name="consts", bufs=1))
        identity = consts.tile([P, P], F16)
        make_identity(nc, identity)
        ones_col = consts.tile([P, 1], F16)
        nc.gpsimd.memset(ones_col, 1.0)

        # Triangular causal mask tri[k, q] = 1 iff q >= k, for the DVE half
        # of the diagonal-tile masking (DVE has no affine_select).
        tri_f32 = consts.tile([P, QC], F32, tag="trif")
        nc.gpsimd.memset(tri_f32, 1.0)
        nc.gpsimd.affine_select(
            out=tri_f32, in_=tri_f32,
            pattern=[[1, QC]],
            compare_op=mybir.AluOpType.is_ge,
            fill=0.0,
            base=0,
            channel_multiplier=-1,
        )
        tri = consts.tile([P, QC], F16, tag="tri")
        nc.vector.tensor_copy(tri, tri_f32)

        # kv weights first (kT/vT projections consume them immediately);
        # q weights queued behind the first x chunk.
        wkv_sb = consts.tile([P, NCC, 2 * HD], F16, tag="wkv")
        wkv_r = wkv_t.rearrange("(cc p) d -> p cc d", p=P)
        for c0 in range(0, NCC, 4):
            nc.sync.dma_start(out=wkv_sb[:, c0:c0 + 4], in_=wkv_r[:, c0:c0 + 4])
        wq_sb = consts.tile([P, NCC, HPC * HD], F16, tag="wq")
        wq_r = wq_t.rearrange("(cc p) d -> p cc d", p=P)

        xt_pool = ctx.enter_context(tc.tile_pool(name="xt", bufs=5))
        kv_pool = ctx.enter_context(tc.tile_pool(name="kv", bufs=1))
        vT_pool = ctx.enter_context(tc.tile_pool(name="vT", bufs=2))
        qT_pool = ctx.enter_context(tc.tile_pool(name="qT", bufs=4))
        pt_pool = ctx.enter_context(tc.tile_pool(name="pt", bufs=5))
        acc_pool = ctx.enter_context(tc.tile_pool(name="acc", bufs=10))
        ysum_pool = ctx.enter_context(tc.tile_pool(name="ysum", bufs=10))
        sums_sb_pool = ctx.enter_context(tc.tile_pool(name="ssb", bufs=3))
        yout_pool = ctx.enter_context(tc.tile_pool(name="yout", bufs=2))
        recip_pool = ctx.enter_context(tc.tile_pool(name="recip", bufs=3))

        # PSUM budget (8 banks): st [128,2,512] x2 bufs = 4, y [128,512] x2
        # = 2, sums [1,512] x2 = 2. Projection accumulators share st's
        # slots; v-transposes, sums-transposes and y-transposes share y's.
        st_pp = ctx.enter_context(tc.tile_pool(name="st_pp", bufs=2,
                                               space="PSUM"))
        y_pp = ctx.enter_context(tc.tile_pool(name="y_pp", bufs=2,
                                              space="PSUM"))
        sums_pp = ctx.enter_context(tc.tile_pool(name="sums_pp", bufs=2,
                                                 space="PSUM"))

        kT = kv_pool.tile([P, T], F16, tag="kT")
        v_sb = kv_pool.tile([P, T // P, HD], F16, tag="v")

        # ---- kv projection for this core's T/4 shard, then all-gather
        # across the 4 same-batch cores (the other 3 shards arrive while
        # the q projections run) ----
        with nc.named_scope("kvshard"):
            xkv_sb = xt_pool.tile([P, NCC, QC], F16, tag="xt")
            xkr = xkv.rearrange("(cc p) t -> p cc t", p=P)
            for c0 in range(0, NCC, 4):
                nc.sync.dma_start(out=xkv_sb[:, c0:c0 + 4],
                                  in_=xkr[:, c0:c0 + 4])
            kloc = kv_pool.tile([P, QC], F16, tag="kloc")
            vloc = kv_pool.tile([P, KTQ, HD], F16, tag="vloc")
            for m in range(2):
                ps = st_pp.tile([P, QC], F32, tag="st")
                for cc in range(NCC):
                    nc.tensor.matmul(
                        ps, lhsT=wkv_sb[:, cc, m * HD:(m + 1) * HD],
                        rhs=xkv_sb[:, cc],
                        start=(cc == 0), stop=(cc == NCC - 1))
                if m == 0:
                    nc.scalar.copy(kloc, ps)
                else:
                    vTl = vT_pool.tile([P, QC], F16, tag="vT")
                    nc.scalar.copy(vTl, ps)
                    for u in range(KTQ):
                        vp = y_pp.tile([P, QC], F16, tag="y")
                        nc.tensor.transpose(vp[:, 0:HD],
                                            vTl[:, u * P:(u + 1) * P],
                                            identity)
                        nc.vector.tensor_copy(vloc[:, u], vp[:, 0:HD])
            nc.sync.dma_start(out=kv_loc[0], in_=kloc)
            nc.sync.dma_start(
                out=kv_loc[1].rearrange("p (tt d) -> p tt d", tt=KTQ),
                in_=vloc)
            nc.gpsimd.collective_compute(
                "AllGather", mybir.AluOpType.bypass,
                replica_groups=[[0, 1, 2, 3], [4, 5, 6, 7]],
                ins=[kv_loc], outs=[kv_all])
            for g2 in range(NB):
                nc.sync.dma_start(out=kT[:, g2 * QC:(g2 + 1) * QC],
                                  in_=kv_all[g2, 0])
                nc.sync.dma_start(
                    out=v_sb[:, g2 * KTQ:(g2 + 1) * KTQ],
                    in_=kv_all[g2, 1].rearrange("p (tt d) -> p tt d", tt=KTQ))

        pending_tails = []

        def emit_tail(tq, ysums, accs):
            # Deferred one chunk: queued behind the next chunk's matmuls so
            # the acc sums and reciprocal are long done when the PE gets
            # here (the sums matmuls never stall on the DVE acc chain).
            with nc.named_scope(f"ltail{tq}"):
                _mark(nc, f"q{tq}:ltail")
                sums2 = sums_sb_pool.tile([NS2, QC], F16, tag="ssb")
                for th in range(HPC):
                    s_ps = sums_pp.tile([1, QC], F32, tag="sums")
                    nc.tensor.matmul(s_ps, lhsT=ones_col, rhs=accs[th],
                                     start=True, stop=True)
                    nc.vector.tensor_copy(sums2[th * 32:th * 32 + 1, :], s_ps)
                rt_ps = y_pp.tile([P, KTQ, NS2], F16, tag="y")
                for qt in range(KTQ):
                    nc.tensor.transpose(rt_ps[:, qt],
                                        sums2[:, qt * P:(qt + 1) * P],
                                        identity[0:NS2, 0:NS2])
                rt = recip_pool.tile([P, KTQ, HPC], F32, tag="recip")
                for th in range(HPC):
                    nc.vector.reciprocal(rt[:, :, th:th + 1],
                                         rt_ps[:, :, th * 32:th * 32 + 1])
                yo = yout_pool.tile([P, KTQ, HPC, HD], F32, tag="yo")
                for th in range(HPC):
                    ytr = y_pp.tile([P, QC], F16, tag="y")
                    for qt in range(KTQ):
                        nc.tensor.transpose(ytr[:, qt * P:(qt + 1) * P],
                                            ysums[th][:, qt * P:(qt + 1) * P],
                                            identity)
                    for qt in range(KTQ):
                        nc.scalar.activation(
                            yo[:, qt, th],
                            ytr[:, qt * P:(qt + 1) * P],
                            mybir.ActivationFunctionType.Copy,
                            scale=rt[:, qt, th:th + 1])
                ydst = y.rearrange("(nq qt p) (hh d) -> nq p qt hh d",
                                   qt=KTQ, p=P, hh=HPC)[tq]
                nc.sync.dma_start(out=ydst, in_=yo)

        wq_loaded = False
        qts = {}

        def emit_proj(tq):
            nonlocal wq_loaded
            _mark(nc, f"q{tq}")
            tslc = slice(tq * QC, (tq + 1) * QC)
            with nc.named_scope(f"load{tq}"):
                xts = xt_pool.tile([P, NCC, QC], F16, tag="xt")
                xr = xT.rearrange("(cc p) t -> p cc t", p=P)
                for c0 in range(0, NCC, 4):
                    nc.sync.dma_start(out=xts[:, c0:c0 + 4],
                                      in_=xr[:, c0:c0 + 4, tslc])
                if not wq_loaded:
                    for c0 in range(0, NCC, 4):
                        nc.sync.dma_start(out=wq_sb[:, c0:c0 + 4],
                                          in_=wq_r[:, c0:c0 + 4])
                    wq_loaded = True

            with nc.named_scope(f"proj{tq}"):
                qTq = qT_pool.tile([P, HPC, QC], F16, tag="qT")
                for h in range(HPC):
                    _mark(nc, f"q{tq}:proj{h}")
                    ps = st_pp.tile([P, QC], F32, tag="st")
                    for cc in range(NCC):
                        nc.tensor.matmul(
                            ps,
                            lhsT=wq_sb[:, cc, h * HD:(h + 1) * HD],
                            rhs=xts[:, cc],
                            start=(cc == 0), stop=(cc == NCC - 1),
                        )
                    nc.scalar.copy(qTq[:, h], ps)
                qts[tq] = qTq

        def emit_attn(tq):
            # ---- causal attention for this query chunk ----
            # Heads run in interleaved PAIRS: the PE alternates score and PV
            # matmuls between the two heads, so each head's exp->mask chain
            # (ACT then DVE) hides under the other head's matmuls.
            last_chunk = tq == NQC - 1
            nkt = (tq + 1) * KTQ
            ngr = nkt // KGRP
            ysums = [None] * HPC
            accs = [None] * HPC
            qTq = qts.pop(tq)

            def s_mm(h, g):
                st = st_pp.tile([P, KGRP, QC], F32, tag="st")
                pt = pt_pool.tile([P, KGRP, QC], F16, tag="pt")
                qrhs = qTq[:, h]
                if g >= 2 * tq:
                    # diagonal group: restrict to q >= du*128, mask
                    for u in range(KGRP):
                        off = (g * KGRP + u - KTQ * tq) * P
                        nc.tensor.matmul(
                            st[:, u, off:],
                            lhsT=kT[:, (g * KGRP + u) * P:(g * KGRP + u + 1) * P],
                            rhs=qrhs[:, off:], start=True, stop=True)
                    for u in range(KGRP):
                        off = (g * KGRP + u - KTQ * tq) * P
                        nc.scalar.activation(
                            pt[:, u, off:], st[:, u, off:],
                            mybir.ActivationFunctionType.Exp,
                            scale=inv_sqrt_hd)
                        nc.vector.tensor_mul(pt[:, u, off:],
                                             pt[:, u, off:],
                                             tri[:, 0:QC - off])
                else:
                    for u in range(KGRP):
                        kt_i = g * KGRP + u
                        nc.tensor.matmul(
                            st[:, u], lhsT=kT[:, kt_i * P:(kt_i + 1) * P],
                            rhs=qrhs, start=True, stop=True)
                    nc.scalar.activation(
                        pt, st, mybir.ActivationFunctionType.Exp,
                        scale=inv_sqrt_hd)
                return pt

            for hp in range(HPC // 2):
              pair = (2 * hp, 2 * hp + 1)
              with nc.named_scope(f"attn{tq}p{hp}"):
                y_psh = {}
                acch = {}
                pts = {}
                for h in pair:
                    y_ps = y_pp.tile([P, QC], F32, tag="y")
                    acc = acc_pool.tile([P, QC], F16, tag="acc")
                    y_psh[h] = y_ps
                    acch[h] = acc
                    pts[h] = {0: s_mm(h, 0)}
                for g in range(ngr):
                    _mark(nc, f"q{tq}:att{hp}g{g}")
                    if g + 1 < ngr:
                        for h in pair:
                            pts[h][g + 1] = s_mm(h, g + 1)
                    for h in pair:
                        pt = pts[h].pop(g)
                        for u in range(KGRP):
                            kt_i = g * KGRP + u
                            off = max(kt_i - KTQ * tq, 0) * P
                            nc.tensor.matmul(
                                y_psh[h][:, off:], lhsT=v_sb[:, kt_i],
                                rhs=pt[:, u, off:],
                                start=(kt_i == 0),
                                stop=(kt_i == nkt - 1),
                                skip_group_check=True)
                        # fp16 running sum of P^T across key tiles (DVE);
                        # feeds the per-head ones-matmul in the tail.
                        if g == 0 and tq > 0:
                            nc.vector.tensor_add(acch[h], pt[:, 0], pt[:, 1])
                        else:
                            for u in range(KGRP):
                                kt_i = g * KGRP + u
                                off = max(kt_i - KTQ * tq, 0) * P
                                if kt_i == 0:
                                    nc.vector.tensor_copy(acch[h], pt[:, u])
                                else:
                                    nc.vector.tensor_add(
                                        acch[h][:, off:], acch[h][:, off:],
                                        pt[:, u, off:])
                _mark(nc, f"q{tq}:tail{hp}")
                for h in pair:
                    ysum = ysum_pool.tile([P, QC], F16, tag="ysum")
                    nc.vector.tensor_copy(ysum, y_psh[h])
                    ysums[h] = ysum
                    accs[h] = acch[h]
                # On the last chunk, drain the previous chunk's tail early
                # (mid-chunk) so only the final tail remains after the loop.
                if last_chunk and hp == 0 and pending_tails:
                    emit_tail(*pending_tails.pop(0))
            pending_tails.append((tq, ysums, accs))
            while len(pending_tails) > (0 if last_chunk else 1):
                emit_tail(*pending_tails.pop(0))

        # All projections run before any attention: ~55us of pure PE work
        # covers the kv all-gather's ~45us trigger-to-data latency, so
        # attn(0) finds kT/v ready (or nearly so).
        for tq in range(NQC):
            emit_proj(tq)
        for tq in range(NQC):
            emit_attn(tq)

    nc.compile()
    return nc


_cache = {}


def _get_nc(T, C):
    key = (T, C)
    if key not in _cache:
        _cache[key] = build_nc(T, C)
    return _cache[key]


def prepare_in_maps(x, w_kv, w_q):
    x = np.asarray(x, dtype=np.float32)
    wkv_t = np.ascontiguousarray(np.asarray(w_kv, np.float32).T).astype(np.float16)
    wq = np.asarray(w_q, dtype=np.float32)
    xTs = [np.ascontiguousarray(x[b].T).astype(np.float16) for b in range(x.shape[0])]
    in_maps = []
    for i in range(N_CORES):
        b, hg = divmod(i, NB)
        wq_sh = np.ascontiguousarray(
            wq[hg * HPC * HD:(hg + 1) * HPC * HD].T).astype(np.float16)
        xkv = np.ascontiguousarray(xTs[b][:, hg * 512:(hg + 1) * 512])
        in_maps.append({"xT": xTs[b], "xkv": xkv, "wq_t": wq_sh,
                        "wkv_t": wkv_t})
    return in_maps


def gather_output(results, B, T, C):
    out = np.empty((B, T, C), np.float32)
    for i in range(N_CORES):
        b, hg = divmod(i, NB)
        out[b, :, hg * HPC * HD:(hg + 1) * HPC * HD] = results[i]["y"]
    return out


def kernel(x, w_kv, w_q):
    x = np.asarray(x)
    B, T, C = x.shape
    nc = _get_nc(T, C)
    in_maps = prepare_in_maps(x, w_kv, w_q)
    res = run_bass_kernel_spmd(nc, in_maps, list(range(N_CORES)))
    return gather_output(res.results, B, T, C)



# revision 6
# speedup vs baseline: 1.1972x; 1.1972x over previous
"""Causal MQA kernel for Trainium2, SPMD over 8 NeuronCores.

Sharding: core i = (batch b = i//4, head-group hg = i%4). Each core computes
K/V projections for its batch locally (no collectives), the q projection for
its 4 heads, and causal attention for those heads; it writes the [T, 512]
fp16 output slice y[b, :, hg*512:(hg+1)*512]. The host concatenates slices.

Device algorithm (per core, T processed in 4 chunks of QC=512 queries):
  - Projections run in fp8e4 with DoubleRow perf mode (2 contraction k-tiles
    per instruction, 0.5 cycles/col): weights are scaled by S=32 on the host
    so fp8 subnormals are avoided; the S^2 factor folds into the exp scale
    and the S factor on V folds into the softmax-denominator constant.
  - Scores S^T[k, q] = matmul(lhsT=kT16 tile, rhs=qT16 chunk) in fp16.
  - P^T = exp(S^T * scale) emitted directly to fp8 (ACT), one instruction
    per key-tile pair; causal masking on diagonal tiles via gpsimd
    affine_select on the [128,128] triangle + a gap memset (both on Pool).
  - PV: y^T accumulates in PSUM via fp8 DoubleRow over key-tile pairs
    (lhsT = v8 pair, rhs = pt8 pair).
  - Softmax denominators: fp8 DoubleRow ones-matmul (lhsT = const 0.5) into
    a [32, 512] PSUM accumulator per head, accumulated across pairs.
  - Early causal rows are noise-sensitive (few keys -> no averaging), so
    chunk 0 (queries 0-511, keys 0-511) runs an accurate path: q/k/v from
    two-term fp8 projections (x8a@w8a + x8a@w8b + x8b@w8a, ~0.5% error) and
    fp16 scores/pt16/PV.
  - tail (deferred one chunk): transpose denominators to [128, qt, h],
    reciprocal across all DVE lanes, PE-transpose y^T, per-partition
    scale-multiply on DVE, one fp16 DMA per chunk.
"""

import math
from contextlib import ExitStack

import numpy as np
import ml_dtypes

import concourse.bass as bass
import concourse.mybir as mybir
import concourse.tile as tile
from concourse import bacc
from concourse.bass_utils import run_bass_kernel_spmd
from concourse.masks import make_identity

F32 = mybir.dt.float32
F16 = mybir.dt.float16
F8 = mybir.dt.float8e4
E4M3 = ml_dtypes.float8_e4m3

P = 128  # partitions
HD = 128  # head dim
QC = 512  # query-chunk width (one fp32 PSUM bank)
N_CORES = 8
HPC = 4  # query heads per core
NB = 4  # head groups (cores per batch)
S = 32.0  # host-side weight scale (fp8 subnormal avoidance)
ALPHA = 1.0 / 64.0  # ysum copy scale (fp16 overflow avoidance)
BETA = S * ALPHA  # denominator matmul constant = 0.5 (exact in fp8)
DR = mybir.MatmulPerfMode.DoubleRow

PHASE_MARKS = []


def _mark(nc, name):
    n = int(nc.get_next_instruction_name().split("-")[-1])
    PHASE_MARKS.append((n, name))


def build_nc(T, C):
    NQC = T // QC  # query chunks (4)
    NCC = C // P  # contraction chunks (16)
    KTQ = QC // P  # key tiles per query chunk (4)
    NKT = T // P  # key tiles total (16)
    NS2 = (HPC - 1) * 32 + 2  # sums rows at h*32 for the tail transpose
    exp_scale = 1.0 / (math.sqrt(HD) * S * S)

    nc = bacc.Bacc("TRN2", target_bir_lowering=False, debug=False,
                   num_devices=N_CORES)
    x8a = nc.dram_tensor("x8a", [C, T], F8, kind="ExternalInput").ap()
    x8b0 = nc.dram_tensor("x8b0", [C, QC], F8, kind="ExternalInput").ap()
    # wq8 = [wq8a | wq8b] along dim1
    wq8 = nc.dram_tensor("wq8", [C, 2 * HPC * HD], F8, kind="ExternalInput").ap()
    # wkv8 = [wk8a | wk8b | wv8a | wv8b]
    wkv8 = nc.dram_tensor("wkv8", [C, 4 * HD], F8, kind="ExternalInput").ap()
    y = nc.dram_tensor("y", [T, HPC * HD], F16, kind="ExternalOutput").ap()

    with tile.TileContext(nc) as tc, ExitStack() as ctx, \
            nc.allow_low_precision(reason="fp8 operands feed the PE; accumulation stays fp32 in PSUM"):
        consts = ctx.enter_context(tc.tile_pool(name="consts", bufs=1))
        identity = consts.tile([P, P], F16)
        make_identity(nc, identity)
        ones8 = consts.tile([P, 2, 32], F8, tag="ones8")
        nc.gpsimd.memset(ones8, BETA)
        ones16 = consts.tile([P, 32], F16, tag="ones16")
        nc.gpsimd.memset(ones16, BETA)

        # ---- persistent SBUF ----
        big = ctx.enter_context(tc.tile_pool(name="big", bufs=1))
        x8a_sb = big.tile([P, NQC, NCC, QC], F8, tag="x8a")
        x8b0_sb = big.tile([P, NCC, QC], F8, tag="x8b0")
        wq8_sb = big.tile([P, NCC, 2 * HPC * HD], F8, tag="wq8")
        wkv8_sb = big.tile([P, NCC, 4 * HD], F8, tag="wkv8")
        kT16 = big.tile([P, T], F16, tag="kT16")
        kT016 = big.tile([P, QC], F16, tag="kT016")
        v8 = big.tile([P, NKT, HD], F8, tag="v8")
        v016 = big.tile([P, KTQ, HD], F16, tag="v016")

        # ---- pools ----
        qT_pool = ctx.enter_context(tc.tile_pool(name="qT", bufs=2))
        pt_pool = ctx.enter_context(tc.tile_pool(name="pt", bufs=6))
        pt16_pool = ctx.enter_context(tc.tile_pool(name="pt16", bufs=3))
        vt_pool = ctx.enter_context(tc.tile_pool(name="vt", bufs=2))
        ysum_pool = ctx.enter_context(tc.tile_pool(name="ysum", bufs=10))
        sums_sb_pool = ctx.enter_context(tc.tile_pool(name="ssb", bufs=3))
        yout_pool = ctx.enter_context(tc.tile_pool(name="yout", bufs=2))
        recip_pool = ctx.enter_context(tc.tile_pool(name="recip", bufs=3))

        # PSUM (8 banks): st 2x[128,2,512] = 4, y 2x[128,512] = 2,
        # sums 2x[32,512] = 2. Projections share st slots; transposes share y.
        st_pp = ctx.enter_context(tc.tile_pool(name="st_pp", bufs=2,
                                               space="PSUM"))
        y_pp = ctx.enter_context(tc.tile_pool(name="y_pp", bufs=2,
                                              space="PSUM"))
        sums_pp = ctx.enter_context(tc.tile_pool(name="sums_pp", bufs=2,
                                                 space="PSUM"))

        # ---- input DMAs ----
        xr = x8a.rearrange("(cc p) t -> p cc t", p=P)
        # startup-critical first: weights for kv, then x chunk 0
        nc.sync.dma_start(out=wkv8_sb,
                          in_=wkv8.rearrange("(cc p) d -> p cc d", p=P))
        nc.sync.dma_start(out=x8a_sb[:, 0], in_=xr[:, :, 0:QC])
        nc.sync.dma_start(out=x8b0_sb,
                          in_=x8b0.rearrange("(cc p) t -> p cc t", p=P))
        # later-needed inputs issue from the (idle-at-startup) ACT queue
        nc.scalar.dma_start(out=wq8_sb,
                            in_=wq8.rearrange("(cc p) d -> p cc d", p=P))
        for tq in range(1, NQC):
            nc.scalar.dma_start(out=x8a_sb[:, tq],
                                in_=xr[:, :, tq * QC:(tq + 1) * QC])

        def dr_proj(ps, w_sl, x_sl, first, last):
            # 8 DoubleRow matmuls: contraction C in pairs of 128-row tiles
            for c4 in range(NCC // 2):
                nc.tensor.matmul(
                    ps, lhsT=w_sl(c4), rhs=x_sl(c4),
                    start=(first and c4 == 0), stop=(last and c4 == NCC // 2 - 1),
                    perf_mode=DR)

        def w_slice(base, h=0):
            # wq8_sb/wkv8_sb slice helper: [:, 2c4:2c4+2, base+h*HD:...]
            def f(src, off):
                return lambda c4: src[:, 2 * c4:2 * c4 + 2, off:off + HD]
            return f(base[0], base[1] + h * HD)

        def x_slice(tq):
            return lambda c4: x8a_sb[:, tq, 2 * c4:2 * c4 + 2]

        def x0b_slice():
            return lambda c4: x8b0_sb[:, 2 * c4:2 * c4 + 2]

        # ---- K projection chunk 0: single-pass + two-term ----
        with nc.named_scope("kproj0"):
            _mark(nc, "kproj0")
            ps = st_pp.tile([P, QC], F32, tag="st")
            dr_proj(ps, w_slice((wkv8_sb, 0)), x_slice(0), True, True)
            nc.vector.tensor_copy(kT16[:, 0:QC], ps)
            ps0 = st_pp.tile([P, QC], F32, tag="st")
            dr_proj(ps0, w_slice((wkv8_sb, 0)), x_slice(0), True, False)
            dr_proj(ps0, w_slice((wkv8_sb, HD)), x_slice(0), False, False)
            dr_proj(ps0, w_slice((wkv8_sb, 0)), x0b_slice(), False, True)
            nc.vector.tensor_copy(kT016, ps0)

        # ---- V projection -> vT16 -> transpose -> v8 (and v016 for chunk0) --
        def v_chunk(tq, two_term):
            ps = st_pp.tile([P, QC], F32, tag="st")
            dr_proj(ps, w_slice((wkv8_sb, 2 * HD)), x_slice(tq), True,
                    not two_term)
            if two_term:
                dr_proj(ps, w_slice((wkv8_sb, 3 * HD)), x_slice(tq), False, False)
                dr_proj(ps, w_slice((wkv8_sb, 2 * HD)), x0b_slice(), False, True)
            vt16 = vt_pool.tile([P, QC], F16, tag="vt")
            nc.vector.tensor_copy(vt16, ps)
            vp = y_pp.tile([P, KTQ, HD], F16, tag="y")
            for u in range(KTQ):
                nc.tensor.transpose(vp[:, u], vt16[:, u * P:(u + 1) * P],
                                    identity)
            if two_term:
                nc.vector.tensor_copy(v016, vp)
            else:
                nc.vector.tensor_copy(v8[:, tq * KTQ:(tq + 1) * KTQ], vp)

        with nc.named_scope("vproj0"):
            _mark(nc, "vproj0")
            v_chunk(0, False)
            v_chunk(0, True)

        # ---- Q projection per (chunk, head) ----
        qts = {}

        def emit_qproj(tq):
            _mark(nc, f"qproj{tq}")
            with nc.named_scope(f"qproj{tq}"):
                qTq = qT_pool.tile([P, HPC, QC], F16, tag="qT")
                for h in range(HPC):
                    ps = st_pp.tile([P, QC], F32, tag="st")
                    dr_proj(ps, w_slice((wq8_sb, 0), h), x_slice(tq),
                            True, tq != 0)
                    if tq == 0:
                        dr_proj(ps, w_slice((wq8_sb, HPC * HD), h), x_slice(0),
                                False, False)
                        dr_proj(ps, w_slice((wq8_sb, 0), h), x0b_slice(),
                                False, True)
                    nc.vector.tensor_copy(qTq[:, h], ps)
                qts[tq] = qTq

        pending_tails = []

        def emit_tail(tq, ysums, sums2):
            # Deferred one chunk: queued behind the next chunk's matmuls.
            with nc.named_scope(f"ltail{tq}"):
                _mark(nc, f"q{tq}:ltail")
                rt_ps = y_pp.tile([P, KTQ, NS2], F16, tag="y")
                for qt in range(KTQ):
                    nc.tensor.transpose(rt_ps[:, qt],
                                        sums2[:, qt * P:(qt + 1) * P],
                                        identity[0:NS2, 0:NS2])
                rt = recip_pool.tile([P, KTQ, HPC], F32, tag="recip")
                for th in range(HPC):
                    nc.vector.reciprocal(rt[:, :, th:th + 1],
                                         rt_ps[:, :, th * 32:th * 32 + 1])
                yo = yout_pool.tile([P, KTQ, HPC, HD], F16, tag="yo")
                for th in range(HPC):
                    ytr = y_pp.tile([P, QC], F16, tag="y")
                    for qt in range(KTQ):
                        nc.tensor.transpose(ytr[:, qt * P:(qt + 1) * P],
                                            ysums[th][:, qt * P:(qt + 1) * P],
                                            identity)
                    for qt in range(KTQ):
                        nc.vector.tensor_scalar_mul(
                            yo[:, qt, th], ytr[:, qt * P:(qt + 1) * P],
                            rt[:, qt, th:th + 1])
                ydst = y.rearrange("(nq qt p) (hh d) -> nq p qt hh d",
                                   qt=KTQ, p=P, hh=HPC)[tq]
                nc.sync.dma_start(out=ydst, in_=yo)

        def emit_attn(tq):
            # ---- causal attention for this query chunk ----
            # Head PAIRS: the PE alternates score and PV matmuls between the
            # two heads so each head's exp chain hides under the other's
            # matmuls. Chunk 0 uses the accurate fp16 path.
            last_chunk = tq == NQC - 1
            fp16_path = tq == 0
            nkt = (tq + 1) * KTQ
            ngr = nkt // 2  # key-tile pairs
            ysums = [None] * HPC
            qTq = qts.pop(tq)
            sums2 = sums_sb_pool.tile([NS2, QC], F16, tag="ssb")
            kt_src = kT016 if fp16_path else kT16

            def s_mm(h, g):
                # scores + exp (+ diag mask) for key-tile pair g
                st = st_pp.tile([P, 2, QC], F32, tag="st")
                if fp16_path:
                    pt = pt16_pool.tile([P, 2, QC], F16, tag="pt16")
                else:
                    pt = pt_pool.tile([P, 2, QC], F8, tag="pt")
                qrhs = qTq[:, h]
                offs = []
                for u in range(2):
                    kt = 2 * g + u
                    off = max((kt - KTQ * tq) * P, 0)
                    offs.append(off)
                    nc.tensor.matmul(
                        st[:, u, off:],
                        lhsT=kt_src[:, kt * P:(kt + 1) * P],
                        rhs=qrhs[:, off:], start=True, stop=True)
                o0, o1 = offs
                nc.scalar.activation(pt[:, :, o0:], st[:, :, o0:],
                                     mybir.ActivationFunctionType.Exp,
                                     scale=exp_scale)
                if 2 * g + 1 >= KTQ * tq:  # pair contains diagonal tiles
                    if o1 > o0:
                        nc.gpsimd.memset(pt[:, 1, o0:o1], 0.0)
                    for u, off in enumerate(offs):
                        nc.gpsimd.affine_select(
                            out=pt[:, u, off:off + P],
                            in_=pt[:, u, off:off + P],
                            pattern=[[1, P]],
                            compare_op=mybir.AluOpType.is_ge,
                            fill=0.0, base=0, channel_multiplier=-1)
                return pt, o0

            for hp in range(HPC // 2):
              pair = (2 * hp, 2 * hp + 1)
              with nc.named_scope(f"attn{tq}p{hp}"):
                y_psh = {}
                s_psh = {}
                pts = {}
                for h in pair:
                    y_ps = y_pp.tile([P, QC], F32, tag="y")
                    y_psh[h] = y_ps
                    s_ps = sums_pp.tile([32, QC], F32, tag="sums")
                    s_psh[h] = s_ps
                    pts[h] = {0: s_mm(h, 0)}
                for g in range(ngr):
                    _mark(nc, f"q{tq}:att{hp}g{g}")
                    if g + 1 < ngr:
                        for h in pair:
                            pts[h][g + 1] = s_mm(h, g + 1)
                    for h in pair:
                        pt, o0 = pts[h].pop(g)
                        first, last = g == 0, g == ngr - 1
                        if fp16_path:
                            for u in range(2):
                                kt = 2 * g + u
                                off = max((kt - KTQ * tq) * P, 0)
                                nc.tensor.matmul(
                                    y_psh[h][:, off:], lhsT=v016[:, kt],
                                    rhs=pt[:, u, off:],
                                    start=(kt == 0), stop=(kt == nkt - 1),
                                    skip_group_check=True)
                                nc.tensor.matmul(
                                    s_psh[h][:, off:], lhsT=ones16,
                                    rhs=pt[:, u, off:],
                                    start=(kt == 0), stop=(kt == nkt - 1),
                                    skip_group_check=True)
                        else:
                            v_sl = v8.rearrange("p (gg two) d -> p gg two d",
                                                two=2)[:, g]
                            nc.tensor.matmul(
                                y_psh[h][:, o0:], lhsT=v_sl,
                                rhs=pt[:, :, o0:],
                                start=first, stop=last, perf_mode=DR,
                                skip_group_check=True)
                            nc.tensor.matmul(
                                s_psh[h][:, o0:], lhsT=ones8,
                                rhs=pt[:, :, o0:],
                                start=first, stop=last, perf_mode=DR,
                                skip_group_check=True)
                _mark(nc, f"q{tq}:tail{hp}")
                for h in pair:
                    ysum = ysum_pool.tile([P, QC], F16, tag="ysum")
                    nc.vector.tensor_scalar_mul(ysum, y_psh[h], ALPHA)
                    ysums[h] = ysum
                    nc.vector.tensor_copy(sums2[h * 32:h * 32 + 1, :],
                                          s_psh[h][0:1, :])
                # On the last chunk, drain the previous chunk's tail early
                if last_chunk and hp == 0 and pending_tails:
                    emit_tail(*pending_tails.pop(0))
            pending_tails.append((tq, ysums, sums2))
            while len(pending_tails) > (0 if last_chunk else 1):
                emit_tail(*pending_tails.pop(0))

        # ---- emission order: chunk 0 projections, attn0, then per-chunk ----
        emit_qproj(0)
        emit_attn(0)
        for tq in range(1, NQC):
            with nc.named_scope(f"kv{tq}"):
                _mark(nc, f"kv{tq}")
                ps = st_pp.tile([P, QC], F32, tag="st")
                dr_proj(ps, w_slice((wkv8_sb, 0)), x_slice(tq), True, True)
                nc.vector.tensor_copy(kT16[:, tq * QC:(tq + 1) * QC], ps)
                v_chunk(tq, False)
            emit_qproj(tq)
            emit_attn(tq)

    nc.compile()
    return nc


_cache = {}


def _get_nc(T, C):
    key = (T, C)
    if key not in _cache:
        _cache[key] = build_nc(T, C)
    return _cache[key]


def prepare_in_maps(x, w_kv, w_q):
    x = np.asarray(x, dtype=np.float32)
    w_kv = np.asarray(w_kv, dtype=np.float32)
    w_q = np.asarray(w_q, dtype=np.float32)
    B, T, C = x.shape

    def two_term(w):  # [C, D] scaled two-term fp8
        ws = np.ascontiguousarray(w) * S
        a = ws.astype(E4M3)
        b = (ws - a.astype(np.float32)).astype(E4M3)
        return a, b

    wk = w_kv[:HD].T  # [C, HD]
    wv = w_kv[HD:].T
    wk8a, wk8b = two_term(wk)
    wv8a, wv8b = two_term(wv)
    wkv8 = np.ascontiguousarray(
        np.concatenate([wk8a, wk8b, wv8a, wv8b], axis=1))

    in_maps = []
    for i in range(N_CORES):
        b, hg = divmod(i, NB)
        xT = np.ascontiguousarray(x[b].T)  # [C, T]
        x8a = xT.astype(E4M3)
        x8b0 = (xT[:, :QC] - x8a[:, :QC].astype(np.float32)).astype(E4M3)
        wqs = w_q[hg * HPC * HD:(hg + 1) * HPC * HD].T  # [C, 512]
        wq8a, wq8b = two_term(wqs)
        wq8 = np.ascontiguousarray(np.concatenate([wq8a, wq8b], axis=1))
        in_maps.append({"x8a": x8a, "x8b0": np.ascontiguousarray(x8b0),
                        "wq8": wq8, "wkv8": wkv8})
    return in_maps


def gather_output(results, B, T, C):
    out = np.empty((B, T, C), np.float32)
    for i in range(N_CORES):
        b, hg = divmod(i, NB)
        out[b, :, hg * HPC * HD:(hg + 1) * HPC * HD] = \
            results[i]["y"].astype(np.float32)
    return out


def kernel(x, w_kv, w_q):
    x = np.asarray(x)
    B, T, C = x.shape
    nc = _get_nc(T, C)
    in_maps = prepare_in_maps(x, w_kv, w_q)
    res = run_bass_kernel_spmd(nc, in_maps, list(range(N_CORES)))
    return gather_output(res.results, B, T, C)
